# revision 46
# baseline (speedup 1.0000x reference)
"""Trainium2 Bass kernel for nn_MultiHeadDensityRatioEstimator (v2, ~268us).

Math restructure vs the jax reference:
  logits l_h(i,j) = -log1p(sq_h(i,j))  with sq = ||zy_i||^2+||zx_j||^2-2<zy_i,zx_j>
  exp(l_h) = 1/(1+sq_h) =: w_h   -> every logsumexp becomes a plain sum of w
  sum_h l_h = ln(prod_h w_h)     -> one log per pair instead of 8

Design (vs the 528us v1 baseline):
  - No collective. Each core exports its per-(row,head) off-diagonal rowsums
    and per-row diagonal logits; the host computes the EXACT logsumexp
    baseline from the 64 global sums, plus exact diag-count/sigmoid stats.
  - On-device sigmoid sweeps use a core-local baseline estimate bl0 (from
    the first 8 j-blocks, err ~2e-3, validated). Count sweeps run at TWO
    bracketing thresholds exp(8*bl0 -/+ 0.06); the host linearly
    interpolates the count at the exact baseline, killing the estimator
    noise that precision/accuracy cannot tolerate.
  - All matmul operands bf16 (1-pass PE, fp32r was 2-pass). Host-side input
    prep is layout/dtype only: bf16 cast, transpose, placement into the
    32-row matmul slots (ones rows prefilled, norm rows zeroed). The -2
    scale and all norms are computed on device.
  - Group-major j-block order (jb = 4k+g): Qst[g] completes every 8 blocks,
    so Ln/sigmoid/count sweeps overlap the main loop; only a quarter-group
    sweep remains in the tail.
  - Per-(i,h) rowsums ride the PE as one-hot E-matmuls accumulating into two
    PSUM banks (A = first 8 blocks -> early bl0; B = rest).
  - Count sweeps use ACT Sign (present in every activation table -> no
    act-table reloads); count = (sum(sign) + total)/2 on the host.
  - Engine balance per j-block: DVE = 3 reciprocals (RECIPROCAL_APPROX_FAST,
    the only fast reciprocal on this chip) + 2 tree muls; GPSIMD = 5 tree
    muls; ACT = sweeps; PE = 8 mains + 8 rowsum matmuls.
"""

import math
import sys

import numpy as np

for _p in ("/opt/trn_rl_repo",):
    if _p not in sys.path:
        sys.path.insert(0, _p)

N = 4096
D = 128
H = 8
DH = 16
NCORES = 8
RPC = N // NCORES  # rows per core = 512
NIB = RPC // 128  # 4 chunks of this core's rows
NJB = N // 128  # 32 j-blocks of 128
NSTAT = 22
LOG_A_PAIRS = float(np.log(512.0 * 1024.0 - 128.0))  # offdiag pairs in A est
CNT_EPS = 0.06  # count-sweep threshold bracket half-width (ln units)


def build_bass():
    import ml_dtypes
    import concourse.bacc as bacc
    import concourse.tile as tile
    from concourse import masks, mybir
    from concourse.dve_ops import RECIP_APPROX_FAST_CONSTS, RECIPROCAL_APPROX_FAST

    f32 = mybir.dt.float32
    bf16 = mybir.dt.bfloat16
    fp16 = mybir.dt.float16
    AF = mybir.ActivationFunctionType
    ALU = mybir.AluOpType
    AX = mybir.AxisListType
    RC = RECIP_APPROX_FAST_CONSTS

    nc = bacc.Bacc("TRN2", num_devices=NCORES, debug=False)

    zxt = nc.dram_tensor("z_xt", [D, N], bf16, kind="ExternalInput")  # zx^T
    zyt = nc.dram_tensor("z_yt", [D, RPC], bf16, kind="ExternalInput")  # zy_c^T
    xta_in = [
        nc.dram_tensor(f"xta{t}", [96, N], bf16, kind="ExternalInput")
        for t in range(3)
    ]
    yta_in = [
        nc.dram_tensor(f"yta{t}", [96, RPC], bf16, kind="ExternalInput")
        for t in range(3)
    ]
    # z_yd[:, 0:128] = this core's zy rows; [:, 128:256] = matching zx rows
    zyd = nc.dram_tensor("z_yd", [RPC, 2 * D], f32, kind="ExternalInput")
    out = nc.dram_tensor("out", [128, NSTAT], f32, kind="ExternalOutput")
    out_rs = nc.dram_tensor("out_rs", [8, RPC], f32, kind="ExternalOutput")

    from contextlib import ExitStack

    # head -> (tensor, slot) packing; matmul operand base partition 0/32/64
    HT = [0, 0, 0, 1, 1, 1, 2, 2]
    HS = [0, 1, 2, 0, 1, 2, 0, 1]
    RHEADS = [[0, 1, 2], [3, 4, 5], [6, 7]]
    ACT_R = 1  # r-group whose reciprocals run on ACT (Ln+Exp)

    with tile.TileContext(nc) as tc, ExitStack() as stk:
        big = stk.enter_context(tc.tile_pool(name="big", bufs=1))
        small = stk.enter_context(tc.tile_pool(name="small", bufs=1))

        # lhsT side (zx): rows [32s,32s+16) = -2*zx_h^T ; 32s+16 = xn_h+0.5 ;
        # 32s+17 = 1.  rhs side (zy): [32s,32s+16) = zy_h^T ; 32s+16 = 1 ;
        # 32s+17 = yn_h+0.5
        XTA = [big.tile([96, N], bf16, tag=f"xta{t}", name=f"XTA{t}") for t in range(3)]
        YTA = [big.tile([96, RPC], bf16, tag=f"yta{t}", name=f"YTA{t}") for t in range(3)]
        # stored q = prod_h w_h (bf16); j-block jb=4k+g lives at
        # Qst[g][:, k*512:(k+1)*512]
        Qst = [big.tile([128, 8 * 512], bf16, tag=f"qst{t}", name=f"Qst{t}") for t in range(4)]

        ident = small.tile([128, 128], f32)
        vdall = small.tile([128, NIB * H], f32)
        wdall = small.tile([128, NIB * H], f32)
        pd1 = small.tile([128, 16], f32)
        pd2 = small.tile([128, 8], f32)
        pdw = small.tile([128, 4], f32)
        stats = small.tile([128, NSTAT], f32)
        slq = small.tile([128, 4], f32)
        ssig = small.tile([128, 4], f32)
        scnt = small.tile([128, 8], f32)  # cols 0:4 lo-thr, 4:8 hi-thr
        slq2 = small.tile([128, 4], f32)
        ssig2 = small.tile([128, 4], f32)
        scnt2 = small.tile([128, 8], f32)
        accq = small.tile([128, 16], f32)
        accq2 = small.tile([128, 16], f32)
        ones128 = small.tile([128, 1], f32)
        ones1 = small.tile([1, 128], f32)
        ones8 = small.tile([8, 1], f32)
        half8 = small.tile([8, 1], f32)
        Eall = small.tile([128, 8 * H], bf16)
        rsS = small.tile([8, RPC], f32)
        wdT = small.tile([8, RPC], f32)
        SA = small.tile([8, 1], f32)
        wdpart = small.tile([8, 1], f32)
        lnSA = small.tile([8, 1], f32)
        bl0 = small.tile([1, 1], f32)
        thr0 = small.tile([1, 2], f32)  # [lo, hi] thresholds
        nbl = small.tile([128, 1], f32)
        nthr_lo = small.tile([128, 1], f32)
        nthr_hi = small.tile([128, 1], f32)
        epslo = small.tile([1, 1], f32)
        epshi = small.tile([1, 1], f32)

        nc.vector.memset(ones128[:], 1.0)
        nc.vector.memset(ones1[:], 1.0)
        nc.vector.memset(ones8[:], 1.0)
        nc.vector.memset(half8[:], 0.5)
        nc.vector.memset(stats[:], 0.0)
        nc.vector.memset(accq[:], 0.0)
        nc.vector.memset(accq2[:], 0.0)
        nc.vector.memset(epslo[:], -CNT_EPS)
        nc.vector.memset(epshi[:], CNT_EPS)
        masks.make_identity(nc, ident[:])

        # E matrix for rowsum matmuls: Eall[:, h*8+a] = (a == h), bf16
        em = np.zeros((128, 8 * H), np.float32)
        for h in range(H):
            em[:, h * 8 + h] = 1.0
        Ed = nc.inline_tensor(em.astype(ml_dtypes.bfloat16), name="eall_const")

        # ---------- preprocessing ----------
        # zx/zy arrive pre-transposed (host-side layout prep): no PE transposes
        with (
            tc.tile_pool(name="pp_sbuf", bufs=4) as pp,
            tc.tile_pool(name="pp_keep", bufs=1) as ppk,
            tc.tile_pool(name="pp_psum", bufs=2, space="PSUM") as ppp,
        ):
            Hmaskb = ppk.tile([128, 8], bf16)
            SXT = ppk.tile([128, N], bf16)   # zx^T
            SYT = ppk.tile([128, RPC], bf16)  # zy_c^T
            SYD = ppk.tile([128, NIB * 2 * D], f32)

            hm = np.zeros((128, 8), np.float32)
            for h in range(H):
                hm[h * DH : (h + 1) * DH, h] = 1.0
            hmd = nc.inline_tensor(hm.astype(ml_dtypes.bfloat16), name="hmask_const")

            nc.sync.dma_start(out=SXT[:], in_=zxt[:])
            nc.scalar.dma_start(out=XTA[0][:], in_=xta_in[0][:])
            nc.sync.dma_start(out=XTA[1][:], in_=xta_in[1][:])
            nc.sync.dma_start(out=XTA[2][:], in_=xta_in[2][:])
            nc.gpsimd.dma_start(out=Hmaskb[:], in_=hmd[:])
            nc.gpsimd.dma_start(out=Eall[:], in_=Ed[:])
            nc.scalar.dma_start(
                out=SYD.rearrange("p (t c) -> p t c", c=2 * D),
                in_=zyd.rearrange("(t p) c -> p t c", p=128),
            )
            nc.gpsimd.dma_start(out=SYT[:], in_=zyt[:])
            for t in range(3):
                nc.gpsimd.dma_start(out=YTA[t][:], in_=yta_in[t][:])

            def SY(t):
                return SYD[:, t * 2 * D : t * 2 * D + D]

            def SXD(t):
                return SYD[:, t * 2 * D + D : (t + 1) * 2 * D]

            # dummy matmul absorbs the staging-DMA wait on PE
            pdm2 = ppp.tile([8, 8], f32, tag="xn")
            nc.tensor.matmul(out=pdm2[:], lhsT=Hmaskb[:, 0:8], rhs=Hmaskb[:, 0:8])

            # xn rows first (they gate the main loop): per-chunk pipeline
            sqx = ppk.tile([128, N], bf16)
            xnab = ppk.tile([8, N], bf16)
            for t in range(N // 512):
                nc.scalar.activation(
                    out=sqx[:, t * 512 : (t + 1) * 512],
                    in_=SXT[:, t * 512 : (t + 1) * 512], func=AF.Square,
                )
                xnp = ppp.tile([8, 512], f32, tag="xn")
                nc.tensor.matmul(
                    out=xnp[:], lhsT=Hmaskb[:, 0:8],
                    rhs=sqx[:, t * 512 : (t + 1) * 512],
                )
                nc.vector.tensor_scalar(
                    out=xnab[:, t * 512 : (t + 1) * 512], in0=xnp[:],
                    scalar1=half8[:], scalar2=None, op0=ALU.add,
                )
            for half in range(2):
                cs, ce = half * (N // 2), (half + 1) * (N // 2)
                for h in range(H):
                    t, sl = HT[h], HS[h]
                    q = nc.gpsimd if h % 2 == 0 else nc.sync
                    q.dma_start(
                        out=XTA[t][32 * sl + 16 : 32 * sl + 17, cs:ce],
                        in_=xnab[h : h + 1, cs:ce],
                    )
            # -2 scale on whole YTA tensors (host prefills ones rows with
            # -0.5 so the scale turns them into +1; yn rows are DMA'd after)
            for t in range(3):
                nc.vector.tensor_scalar(
                    out=YTA[t][:], in0=YTA[t][:],
                    scalar1=-2.0, scalar2=None, op0=ALU.mult,
                )
            # yn rows
            sqy = pp.tile([128, RPC], bf16, tag="sqy")
            nc.scalar.activation(out=sqy[:], in_=SYT[:], func=AF.Square)
            ynab = ppk.tile([8, RPC], bf16)
            for t in range(RPC // 512):
                ynp = ppp.tile([8, 512], f32, tag="xn")
                nc.tensor.matmul(
                    out=ynp[:], lhsT=Hmaskb[:, 0:8],
                    rhs=sqy[:, t * 512 : (t + 1) * 512],
                )
                nc.vector.tensor_scalar(
                    out=ynab[:, t * 512 : (t + 1) * 512], in0=ynp[:],
                    scalar1=half8[:], scalar2=None, op0=ALU.add,
                )
            for h in range(H):
                t, sl = HT[h], HS[h]
                q = nc.gpsimd if h % 2 == 0 else nc.sync
                q.dma_start(
                    out=YTA[t][32 * sl + 17 : 32 * sl + 18, :],
                    in_=ynab[h : h + 1, :],
                )


        def emit_diag():
            # diagonal path (f32): vd_h(i) = 1 + ||zy_i - zx_i||^2 per head;
            # emitted inside the loop so it fills DVE gaps instead of
            # blocking the first reciprocals at the queue head
            for t in range(NIB):
                dd = dgp.tile([128, 128], f32, tag="dd")
                nc.vector.tensor_sub(dd[:], SYD[:, t * 256 : t * 256 + 128],
                                     SYD[:, t * 256 + 128 : (t + 1) * 256])
                nc.vector.tensor_mul(dd[:], dd[:], dd[:])
                nc.vector.tensor_reduce(
                    out=vdall[:, t * H : (t + 1) * H],
                    in_=dd.rearrange("p (h k) -> p h k", k=DH),
                    axis=AX.X, op=ALU.add,
                )
            nc.vector.tensor_scalar(
                out=vdall[:], in0=vdall[:], scalar1=1.0, scalar2=None, op0=ALU.add
            )
            nc.vector.reciprocal_approx_fast(out=wdall[:], in_=vdall[:])
            wv = wdall.rearrange("p (t c) -> p t c", c=8)
            nc.vector.tensor_mul(
                pd1.rearrange("p (t c) -> p t c", c=4), wv[:, :, 0:4], wv[:, :, 4:8]
            )
            p1v = pd1.rearrange("p (t c) -> p t c", c=4)
            nc.vector.tensor_mul(
                pd2.rearrange("p (t c) -> p t c", c=2), p1v[:, :, 0:2], p1v[:, :, 2:4]
            )
            p2v = pd2.rearrange("p (t c) -> p t c", c=2)
            nc.vector.tensor_mul(
                pdw.rearrange("p (t c) -> p t c", c=1), p2v[:, :, 0:1], p2v[:, :, 1:2]
            )
            # Ld export (stats cols 0:4)
            nc.scalar.activation(out=stats[:, 0:4], in_=pdw[:], func=AF.Ln)
            # wdT: diag w transposed to [8, RPC] (for rowsum subtraction)
            for t in range(NIB):
                ptw = aux.tile([128, 128], f32, tag="aux")
                nc.tensor.transpose(
                    ptw[0:8, :], wdall[:, t * 8 : (t + 1) * 8], ident[:]
                )
                nc.scalar.activation(
                    out=wdT[:, t * 128 : (t + 1) * 128], in_=ptw[0:8, :],
                    func=AF.Copy,
                )

        # ---------- main loop (group-major: jb = 4k+g) ----------
        dgp = stk.enter_context(tc.tile_pool(name="diag_sbuf", bufs=2))
        rp = stk.enter_context(tc.tile_pool(name="rs_psum", bufs=1, space="PSUM"))
        aux = stk.enter_context(tc.tile_pool(name="aux_psum", bufs=1, space="PSUM"))
        rsAB = rp.tile([40, 512], f32)  # A at partitions 0:8, B at 32:40
        with (
            tc.tile_pool(name="mm_psum", bufs=2, space="PSUM") as mp,
            tc.tile_pool(name="wpool2", bufs=6) as wp2,
            tc.tile_pool(name="lnvp", bufs=5) as lvp,
            tc.tile_pool(name="upool", bufs=12) as up,
            tc.tile_pool(name="qpool", bufs=3) as qp,
            tc.tile_pool(name="lnqp", bufs=2) as lqp,
            tc.tile_pool(name="junkp", bufs=2) as jp,
        ):
            def sweeps(g, half, q0=0, q1=4):
                """Ln + sigmoid + count sweeps over chunks [q0,q1) of a half
                of Qst[g] (chunk = 512 cols)."""
                base = half * 4 * 512
                qs = Qst[g][:, base + q0 * 512 : base + q1 * 512]
                W = (q1 - q0) * 512
                lnq = lqp.tile([128, W], bf16, tag="lnq")
                if q0 == 0:
                    a_ln = (slq if half == 0 else slq2)[:, g : g + 1]
                    a_sg = (ssig if half == 0 else ssig2)[:, g : g + 1]
                    a_lo = (scnt if half == 0 else scnt2)[:, g : g + 1]
                    a_hi = (scnt if half == 0 else scnt2)[:, 4 + g : 5 + g]
                else:
                    aq = accq if q0 == 2 else accq2
                    a_ln = aq[:, g : g + 1]
                    a_sg = aq[:, 4 + g : 5 + g]
                    a_lo = aq[:, 8 + g : 9 + g]
                    a_hi = aq[:, 12 + g : 13 + g]
                nc.scalar.activation(
                    out=lnq[:], in_=qs, func=AF.Ln, accum_out=a_ln,
                )
                sj = jp.tile([128, W], bf16, tag="sj")
                nc.scalar.activation(
                    out=sj[:], in_=lnq[:], func=AF.Sigmoid, scale=1.0 / H,
                    bias=nbl[:], accum_out=a_sg,
                )
                cj = jp.tile([128, W], bf16, tag="sj")
                nc.scalar.activation(
                    out=cj[:], in_=qs, func=AF.Sign, bias=nthr_lo[:],
                    accum_out=a_lo,
                )
                ch = jp.tile([128, W], bf16, tag="sj")
                nc.scalar.activation(
                    out=ch[:], in_=qs, func=AF.Sign, bias=nthr_hi[:],
                    accum_out=a_hi,
                )

            for e in range(NJB):
                g, k = e // 8, e % 8
                jb = 4 * k + g
                acc = rsAB[0:8, :] if g == 0 else rsAB[32:40, :]
                w2t = []
                for r, heads in enumerate(RHEADS):
                    L = len(heads) * 512
                    w2 = wp2.tile([128, L], bf16, tag=f"w2{r}", name=f"w2_{r}")
                    ps = mp.tile([128, 1536], f32, tag="ps")
                    for si, h in enumerate(heads):
                        nc.tensor.matmul(
                            out=ps[:, si * 512 : (si + 1) * 512],
                            lhsT=XTA[r][32 * si : 32 * si + 18,
                                        jb * 128 : (jb + 1) * 128],
                            rhs=YTA[r][32 * si : 32 * si + 18, :],
                        )
                    nc.vector._custom_dve(
                        RECIPROCAL_APPROX_FAST,
                        out=w2[:], in0=ps[:, 0:L],
                        s0=RC["s0"], s1=RC["s1"], imm2=RC["imm2"],
                    )
                    # per-(i,h) row sums accumulate on the PE
                    for si, h in enumerate(heads):
                        nc.tensor.matmul(
                            out=acc[:],
                            lhsT=Eall[:, h * 8 : (h + 1) * 8],
                            rhs=w2[:, si * 512 : (si + 1) * 512],
                            start=(e in (0, 8) and h == 0),
                            stop=(e in (7, 31) and h == H - 1),
                            skip_group_check=True,
                        )
                    w2t.append(w2)
                # product tree over the 8 heads in bf16
                pairs = [
                    (w2t[0][:, 0:512], w2t[0][:, 512:1024]),      # h0*h1
                    (w2t[0][:, 1024:1536], w2t[1][:, 0:512]),     # h2*h3
                    (w2t[1][:, 512:1024], w2t[1][:, 1024:1536]),  # h4*h5
                    (w2t[2][:, 0:512], w2t[2][:, 512:1024]),      # h6*h7
                ]
                us = []
                for pi, (a, b) in enumerate(pairs):
                    u = up.tile([128, 512], bf16, tag="u", name=f"u{pi}")
                    if pi < 2:
                        nc.vector.tensor_mul(u[:], a, b)
                    else:
                        nc.gpsimd.tensor_mul(u[:], a, b)
                    us.append(u)
                qa = qp.tile([128, 512], bf16, tag="q")
                qb = qp.tile([128, 512], bf16, tag="q")
                nc.gpsimd.tensor_mul(qa[:], us[0][:], us[1][:])
                nc.gpsimd.tensor_mul(qb[:], us[2][:], us[3][:])
                nc.gpsimd.tensor_mul(
                    Qst[g][:, k * 512 : (k + 1) * 512], qa[:], qb[:]
                )
                if e == 2:
                    emit_diag()
                if e == 7:
                    # bl0 estimate from accumulator A (j-blocks {0,4,..,28};
                    # diag elements only for local rows 0:127)
                    nc.vector.tensor_reduce(
                        out=SA[:], in_=rsAB[0:8, :], axis=AX.X, op=ALU.add
                    )
                    nc.vector.tensor_reduce(
                        out=wdpart[:], in_=wdT[:, 0:128], axis=AX.X, op=ALU.add
                    )
                    nc.vector.tensor_sub(SA[:], SA[:], wdpart[:])
                    nc.scalar.activation(out=lnSA[:], in_=SA[:], func=AF.Ln)
                    ps1 = aux.tile([1, 1], f32, tag="aux")
                    nc.tensor.matmul(out=ps1[:], lhsT=ones8[:, 0:1], rhs=lnSA[:])
                    nc.scalar.activation(
                        out=bl0[:], in_=ps1[:], func=AF.Copy, scale=1.0 / H,
                        bias=-LOG_A_PAIRS,
                    )
                    # bracket thresholds exp(H*bl0 -/+ eps); host interpolates
                    # the count at the exact baseline between them
                    nc.scalar.activation(
                        out=thr0[:, 0:1], in_=bl0[:], func=AF.Exp, scale=float(H),
                        bias=epslo[:],
                    )
                    nc.scalar.activation(
                        out=thr0[:, 1:2], in_=bl0[:], func=AF.Exp, scale=float(H),
                        bias=epshi[:],
                    )
                    psB_ = aux.tile([128, 1], f32, tag="aux")
                    nc.tensor.matmul(out=psB_[:], lhsT=ones1[0:1, :], rhs=bl0[0:1, :])
                    nc.scalar.activation(
                        out=nbl[:], in_=psB_[:], func=AF.Copy, scale=-1.0
                    )
                    psT_ = aux.tile([128, 2], f32, tag="aux")
                    nc.tensor.matmul(out=psT_[:], lhsT=ones1[0:1, :], rhs=thr0[0:1, :])
                    nc.scalar.activation(
                        out=nthr_lo[:], in_=psT_[:, 0:1], func=AF.Copy, scale=-1.0
                    )
                    nc.scalar.activation(
                        out=nthr_hi[:], in_=psT_[:, 1:2], func=AF.Copy, scale=-1.0
                    )
                    # SA export + bl0 export (host reproduces the thresholds)
                    nc.vector.tensor_copy(stats[0:8, 20:21], SA[:])
                    nc.vector.tensor_copy(stats[0:1, 21:22], bl0[:])
                    # pre-subtract: rsS = A - wdT (tail just adds B)
                    nc.scalar.activation(out=rsS[:], in_=rsAB[0:8, :], func=AF.Copy)
                    nc.vector.tensor_sub(rsS[:], rsS[:], wdT[:])
                if k == 3 and e >= 11:
                    sweeps(g, 0)
                if e == 29:
                    sweeps(3, 1, 0, 2)   # quarter/eighth splits: last group
                if e == 30:
                    sweeps(3, 1, 2, 3)
                if k == 7:
                    if e == 7:
                        sweeps(0, 0)
                    if e == 31:
                        sweeps(3, 1, 3, 4)
                    else:
                        sweeps(g, 1)

        # ---------- finish ----------
        with (
            tc.tile_pool(name="fin_sbuf", bufs=2) as fs,
        ):
            # rsS already holds A - wdT; add B
            nc.vector.tensor_add(rsS[:], rsS[:], rsAB[32:40, :])
            # exports
            nc.sync.dma_start(out=out_rs[:], in_=rsS[:])
            nc.vector.tensor_add(stats[:, 4:8], slq[:], slq2[:])
            nc.vector.tensor_add(stats[:, 4:8], stats[:, 4:8], accq[:, 0:4])
            nc.vector.tensor_add(stats[:, 4:8], stats[:, 4:8], accq2[:, 0:4])
            nc.vector.tensor_add(stats[:, 8:12], ssig[:], ssig2[:])
            nc.vector.tensor_add(stats[:, 8:12], stats[:, 8:12], accq[:, 4:8])
            nc.vector.tensor_add(stats[:, 8:12], stats[:, 8:12], accq2[:, 4:8])
            nc.vector.tensor_add(stats[:, 12:16], scnt[:, 0:4], scnt2[:, 0:4])
            nc.vector.tensor_add(stats[:, 12:16], stats[:, 12:16], accq[:, 8:12])
            nc.vector.tensor_add(stats[:, 12:16], stats[:, 12:16], accq2[:, 8:12])
            nc.vector.tensor_add(stats[:, 16:20], scnt[:, 4:8], scnt2[:, 4:8])
            nc.vector.tensor_add(stats[:, 16:20], stats[:, 16:20], accq[:, 12:16])
            nc.vector.tensor_add(stats[:, 16:20], stats[:, 16:20], accq2[:, 12:16])
            nc.sync.dma_start(out=out[:], in_=stats[:])

    nc.compile()
    return nc


_CACHED_NC = None


def _get_nc():
    global _CACHED_NC
    if _CACHED_NC is None:
        _CACHED_NC = build_bass()
    return _CACHED_NC


_HT = [0, 0, 0, 1, 1, 1, 2, 2]
_HS = [0, 1, 2, 0, 1, 2, 0, 1]


def make_in_maps(z_x, z_y):
    """Host-side prep is layout + dtype only: shard, transpose, and place the
    z rows into the 32-row matmul slots (constant one-rows prefilled, norm
    rows zeroed -- the device computes all arithmetic: -2 scale, norms)."""
    import ml_dtypes

    bf = ml_dtypes.bfloat16
    z_x32 = np.ascontiguousarray(z_x, dtype=np.float32)
    z_y32 = np.ascontiguousarray(z_y, dtype=np.float32)
    z_xt = np.ascontiguousarray(z_x32.astype(bf).T)
    z_yt_full = z_y32.astype(bf).T
    xta = [np.zeros((96, N), bf) for t in range(3)]
    for h in range(H):
        t, s = _HT[h], _HS[h]
        xta[t][32 * s : 32 * s + 16] = z_xt[DH * h : DH * (h + 1)]
        xta[t][32 * s + 17] = np.ones((N,), bf)
    maps = []
    for c in range(NCORES):
        z_yt = np.ascontiguousarray(z_yt_full[:, c * RPC : (c + 1) * RPC])
        yta = [np.zeros((96, RPC), bf) for t in range(3)]
        for h in range(H):
            t, s = _HT[h], _HS[h]
            yta[t][32 * s : 32 * s + 16] = z_yt[DH * h : DH * (h + 1)]
            yta[t][32 * s + 16] = np.full((RPC,), -0.5, bf)
        maps.append(
            {
                "z_xt": z_xt,
                "z_yt": z_yt,
                "xta0": xta[0], "xta1": xta[1], "xta2": xta[2],
                "yta0": yta[0], "yta1": yta[1], "yta2": yta[2],
                "z_yd": np.ascontiguousarray(
                    np.concatenate(
                        [
                            z_y32[c * RPC : (c + 1) * RPC],
                            z_x32[c * RPC : (c + 1) * RPC],
                        ],
                        axis=1,
                    )
                ),
            }
        )
    return maps


def combine(stats, z_x, z_y):
    """stats: [NCORES, 128, NSTAT] float; returns the 9 reference outputs.

    stats = (st [NCORES,128,17], rs_all [NCORES,8,RPC]).
    st cols 0:4 Ld[p,t]; 4:8 slq; 8:12 ssig; 12:16 scnt;
    col 16 parts 0:8 = SA_h (partial sums the device used for bl0/thr0).
    """
    st, rs_all = stats
    st = st.astype(np.float64)
    Ld = st[:, :, 0:4]                       # [c, p, t]
    slq = st[:, :, 4:8].sum()
    ssig = st[:, :, 8:12].sum()
    swept = 128.0 * 4096.0 * 4                  # values per core per bracket
    cnt_lo = (st[:, :, 12:16].sum(axis=(1, 2)) + swept) / 2.0  # [c]
    cnt_hi = (st[:, :, 16:20].sum(axis=(1, 2)) + swept) / 2.0  # [c]
    rs = rs_all.astype(np.float64)           # [c, h, i]
    SA = st[:, 0:8, 20]                      # [c, h]
    bl0_dev = st[:, 0, 21]                   # [c] the bl0 each device used

    S_h = rs.sum(axis=(0, 2))                # [h]
    blavg = np.log(S_h).mean() - math.log(float(N) * (N - 1))
    rep_sum = np.log(rs).sum()

    sum_Ld = Ld.sum()
    Ld_flat = Ld.reshape(NCORES, -1)
    # exact diag stats with exact blavg
    cp = float((Ld_flat / H - blavg > 0).sum())
    sig_diag = (1.0 / (1.0 + np.exp(-(Ld_flat / H - blavg)))).sum()
    # device-side sigmoid model (bl0 per core, exported)
    sig_diag_dev = 0.0
    for c in range(NCORES):
        sig_diag_dev += (1.0 / (1.0 + np.exp(-(Ld_flat[c] / H - bl0_dev[c])))).sum()
    # count at the EXACT threshold t=8*blavg by linear interpolation between
    # the two bracketing device sweeps at 8*bl0_dev -/+ CNT_EPS
    t_true = H * blavg
    t_lo = H * bl0_dev - CNT_EPS             # [c]
    frac = (t_true - t_lo) / (2.0 * CNT_EPS)
    cnt_at_true = cnt_lo + (cnt_hi - cnt_lo) * frac  # counts decrease in t
    cnt_full = cnt_at_true.sum()

    mean_pos = sum_Ld / (H * N) - blavg
    mean_neg = (slq - sum_Ld) / (H * N * (N - 1)) - blavg
    mean_sig_pos = sig_diag / N
    mean_sig_neg = (ssig - sig_diag_dev) / (N * (N - 1))
    cn = cnt_full - cp
    acc = (cp + (N * (N - 1) - cn)) / (N * N)
    recall = cp / N
    tpfp = cp + cn
    precision = (cp / max(tpfp, 1.0)) if tpfp > 0 else 0.0
    rep_mean = rep_sum / (H * N) - math.log(N - 1) - blavg
    zx64 = np.asarray(z_x, np.float64)
    zy64 = np.asarray(z_y, np.float64)
    decay = 0.01 * (np.mean(zx64 * zx64) + np.mean(zy64 * zy64))
    loss = -mean_pos + rep_mean + decay
    return np.array(
        [
            mean_pos, mean_neg, mean_sig_pos, mean_sig_neg, acc, recall,
            precision, blavg, loss,
        ],
        dtype=np.float32,
    )


def run_on_hw(z_x, z_y, trace=False):
    from concourse.bass_utils import run_bass_kernel_spmd

    nc = _get_nc()
    res = run_bass_kernel_spmd(
        nc, make_in_maps(z_x, z_y), core_ids=list(range(NCORES)), trace=trace
    )
    st = np.stack([np.asarray(r["out"]) for r in res.results])
    rs_all = np.stack([np.asarray(r["out_rs"]) for r in res.results])
    return combine((st, rs_all), z_x, z_y), res


def kernel(z_x, z_y):
    out, _ = run_on_hw(z_x, z_y, trace=False)
    return out


# revision 47
# speedup vs baseline: 1.1928x; 1.1928x over previous
"""Trainium2 Bass kernel for nn_MultiHeadDensityRatioEstimator (v2, ~268us).

Math restructure vs the jax reference:
  logits l_h(i,j) = -log1p(sq_h(i,j))  with sq = ||zy_i||^2+||zx_j||^2-2<zy_i,zx_j>
  exp(l_h) = 1/(1+sq_h) =: w_h   -> every logsumexp becomes a plain sum of w
  sum_h l_h = ln(prod_h w_h)     -> one log per pair instead of 8

Design (vs the 528us v1 baseline):
  - No collective. Each core exports its per-(row,head) off-diagonal rowsums
    and per-row diagonal logits; the host computes the EXACT logsumexp
    baseline from the 64 global sums, plus exact diag-count/sigmoid stats.
  - On-device sigmoid sweeps use a core-local baseline estimate bl0 (from
    the first 8 j-blocks, err ~2e-3, validated). Count sweeps run at TWO
    bracketing thresholds exp(8*bl0 -/+ 0.06); the host linearly
    interpolates the count at the exact baseline, killing the estimator
    noise that precision/accuracy cannot tolerate.
  - All matmul operands bf16 (1-pass PE, fp32r was 2-pass). Host-side input
    prep is layout/dtype only: bf16 cast, transpose, placement into the
    32-row matmul slots (ones rows prefilled, norm rows zeroed). The -2
    scale and all norms are computed on device.
  - Group-major j-block order (jb = 4k+g): Qst[g] completes every 8 blocks,
    so Ln/sigmoid/count sweeps overlap the main loop; only a quarter-group
    sweep remains in the tail.
  - Per-(i,h) rowsums ride the PE as one-hot E-matmuls accumulating into two
    PSUM banks (A = first 8 blocks -> early bl0; B = rest).
  - Count sweeps use ACT Sign (present in every activation table -> no
    act-table reloads); count = (sum(sign) + total)/2 on the host.
  - Engine balance per j-block: DVE = 3 reciprocals (RECIPROCAL_APPROX_FAST,
    the only fast reciprocal on this chip) + 2 tree muls; GPSIMD = 5 tree
    muls; ACT = sweeps; PE = 8 mains + 8 rowsum matmuls.
"""

import math
import sys

import numpy as np

for _p in ("/opt/trn_rl_repo",):
    if _p not in sys.path:
        sys.path.insert(0, _p)

N = 4096
D = 128
H = 8
DH = 16
NCORES = 8
RPC = N // NCORES  # rows per core = 512
NIB = RPC // 128  # 4 chunks of this core's rows
NJB = N // 128  # 32 j-blocks of 128
NSTAT = 22
LOG_A_PAIRS = float(np.log(512.0 * 1024.0 - 128.0))  # offdiag pairs in A est
CNT_EPS = 0.06  # count-sweep threshold bracket half-width (ln units)


def build_bass():
    import ml_dtypes
    import concourse.bacc as bacc
    import concourse.tile as tile
    from concourse import masks, mybir
    from concourse.dve_ops import RECIP_APPROX_FAST_CONSTS, RECIPROCAL_APPROX_FAST

    f32 = mybir.dt.float32
    bf16 = mybir.dt.bfloat16
    fp16 = mybir.dt.float16
    AF = mybir.ActivationFunctionType
    ALU = mybir.AluOpType
    AX = mybir.AxisListType
    RC = RECIP_APPROX_FAST_CONSTS

    nc = bacc.Bacc("TRN2", num_devices=NCORES, debug=False)

    zxt = nc.dram_tensor("z_xt", [D, N], bf16, kind="ExternalInput")  # zx^T
    zyt = nc.dram_tensor("z_yt", [D, RPC], bf16, kind="ExternalInput")  # zy_c^T
    xta_in = [
        nc.dram_tensor(f"xta{t}", [96, N], bf16, kind="ExternalInput")
        for t in range(3)
    ]
    yta_in = [
        nc.dram_tensor(f"yta{t}", [96, RPC], bf16, kind="ExternalInput")
        for t in range(3)
    ]
    # z_yd[:, 0:128] = this core's zy rows; [:, 128:256] = matching zx rows
    zyd = nc.dram_tensor("z_yd", [RPC, 2 * D], f32, kind="ExternalInput")
    out = nc.dram_tensor("out", [128, NSTAT], f32, kind="ExternalOutput")
    out_rs = nc.dram_tensor("out_rs", [8, RPC], f32, kind="ExternalOutput")

    from contextlib import ExitStack

    # head -> (tensor, slot) packing; matmul operand base partition 0/32/64
    HT = [0, 0, 0, 1, 1, 1, 2, 2]
    HS = [0, 1, 2, 0, 1, 2, 0, 1]
    RHEADS = [[0, 1, 2], [3, 4, 5], [6, 7]]
    ACT_R = 1  # r-group whose reciprocals run on ACT (Ln+Exp)

    with tile.TileContext(nc) as tc, ExitStack() as stk:
        big = stk.enter_context(tc.tile_pool(name="big", bufs=1))
        small = stk.enter_context(tc.tile_pool(name="small", bufs=1))

        # lhsT side (zx): rows [32s,32s+16) = -2*zx_h^T ; 32s+16 = xn_h+0.5 ;
        # 32s+17 = 1.  rhs side (zy): [32s,32s+16) = zy_h^T ; 32s+16 = 1 ;
        # 32s+17 = yn_h+0.5
        XTA = [big.tile([96, N], bf16, tag=f"xta{t}", name=f"XTA{t}") for t in range(3)]
        YTA = [big.tile([96, RPC], bf16, tag=f"yta{t}", name=f"YTA{t}") for t in range(3)]
        # stored q = prod_h w_h (bf16); j-block jb=4k+g lives at
        # Qst[g][:, k*512:(k+1)*512]
        Qst = [big.tile([128, 8 * 512], bf16, tag=f"qst{t}", name=f"Qst{t}") for t in range(4)]

        ident = small.tile([128, 128], f32)
        vdall = small.tile([128, NIB * H], f32)
        wdall = small.tile([128, NIB * H], f32)
        pd1 = small.tile([128, 16], f32)
        pd2 = small.tile([128, 8], f32)
        pdw = small.tile([128, 4], f32)
        stats = small.tile([128, NSTAT], f32)
        slq = small.tile([128, 4], f32)
        ssig = small.tile([128, 4], f32)
        scnt = small.tile([128, 8], f32)  # cols 0:4 lo-thr, 4:8 hi-thr
        slq2 = small.tile([128, 4], f32)
        ssig2 = small.tile([128, 4], f32)
        scnt2 = small.tile([128, 8], f32)
        accq = small.tile([128, 16], f32)
        accq2 = small.tile([128, 16], f32)
        ones128 = small.tile([128, 1], f32)
        ones1 = small.tile([1, 128], f32)
        ones8 = small.tile([8, 1], f32)
        half8 = small.tile([8, 1], f32)
        Eall = small.tile([128, 8 * H], bf16)
        rsS = small.tile([8, RPC], f32)
        wdT = small.tile([8, RPC], f32)
        SA = small.tile([8, 1], f32)
        wdpart = small.tile([8, 1], f32)
        lnSA = small.tile([8, 1], f32)
        bl0 = small.tile([1, 1], f32)
        thr0 = small.tile([1, 2], f32)  # [lo, hi] thresholds
        nbl = small.tile([128, 1], f32)
        nthr_lo = small.tile([128, 1], f32)
        nthr_hi = small.tile([128, 1], f32)
        epslo = small.tile([1, 1], f32)
        epshi = small.tile([1, 1], f32)

        nc.vector.memset(ones128[:], 1.0)
        nc.vector.memset(ones1[:], 1.0)
        nc.vector.memset(ones8[:], 1.0)
        nc.vector.memset(half8[:], 0.5)
        nc.vector.memset(stats[:], 0.0)
        nc.vector.memset(accq[:], 0.0)
        nc.vector.memset(accq2[:], 0.0)
        nc.vector.memset(epslo[:], -CNT_EPS)
        nc.vector.memset(epshi[:], CNT_EPS)
        masks.make_identity(nc, ident[:])

        # E matrix for rowsum matmuls: Eall[:, h*8+a] = (a == h), bf16
        em = np.zeros((128, 8 * H), np.float32)
        for h in range(H):
            em[:, h * 8 + h] = 1.0
        Ed = nc.inline_tensor(em.astype(ml_dtypes.bfloat16), name="eall_const")

        # ---------- preprocessing ----------
        # zx/zy arrive pre-transposed (host-side layout prep): no PE transposes
        with (
            tc.tile_pool(name="pp_sbuf", bufs=4) as pp,
            tc.tile_pool(name="pp_keep", bufs=1) as ppk,
            tc.tile_pool(name="pp_psum", bufs=2, space="PSUM") as ppp,
        ):
            Hmaskb = ppk.tile([128, 8], bf16)
            SXT = ppk.tile([128, N], bf16)   # zx^T
            SYT = ppk.tile([128, RPC], bf16)  # zy_c^T
            SYD = ppk.tile([128, NIB * 2 * D], f32)

            hm = np.zeros((128, 8), np.float32)
            for h in range(H):
                hm[h * DH : (h + 1) * DH, h] = 1.0
            hmd = nc.inline_tensor(hm.astype(ml_dtypes.bfloat16), name="hmask_const")

            nc.sync.dma_start(out=SXT[:], in_=zxt[:])
            nc.scalar.dma_start(out=XTA[0][:], in_=xta_in[0][:])
            nc.sync.dma_start(out=XTA[1][:], in_=xta_in[1][:])
            nc.sync.dma_start(out=XTA[2][:], in_=xta_in[2][:])
            nc.gpsimd.dma_start(out=Hmaskb[:], in_=hmd[:])
            nc.gpsimd.dma_start(out=Eall[:], in_=Ed[:])
            nc.scalar.dma_start(
                out=SYD.rearrange("p (t c) -> p t c", c=2 * D),
                in_=zyd.rearrange("(t p) c -> p t c", p=128),
            )
            nc.gpsimd.dma_start(out=SYT[:], in_=zyt[:])
            for t in range(3):
                nc.gpsimd.dma_start(out=YTA[t][:], in_=yta_in[t][:])

            def SY(t):
                return SYD[:, t * 2 * D : t * 2 * D + D]

            def SXD(t):
                return SYD[:, t * 2 * D + D : (t + 1) * 2 * D]

            # dummy matmul absorbs the staging-DMA wait on PE
            pdm2 = ppp.tile([8, 8], f32, tag="xn")
            nc.tensor.matmul(out=pdm2[:], lhsT=Hmaskb[:, 0:8], rhs=Hmaskb[:, 0:8])

            # xn rows first (they gate the main loop): per-chunk pipeline
            sqx = ppk.tile([128, N], bf16)
            xnab = ppk.tile([8, N], bf16)
            for t in range(N // 512):
                nc.scalar.activation(
                    out=sqx[:, t * 512 : (t + 1) * 512],
                    in_=SXT[:, t * 512 : (t + 1) * 512], func=AF.Square,
                )
                xnp = ppp.tile([8, 512], f32, tag="xn")
                nc.tensor.matmul(
                    out=xnp[:], lhsT=Hmaskb[:, 0:8],
                    rhs=sqx[:, t * 512 : (t + 1) * 512],
                )
                nc.vector.tensor_scalar(
                    out=xnab[:, t * 512 : (t + 1) * 512], in0=xnp[:],
                    scalar1=half8[:], scalar2=None, op0=ALU.add,
                )
            for half in range(2):
                cs, ce = half * (N // 2), (half + 1) * (N // 2)
                for h in range(H):
                    t, sl = HT[h], HS[h]
                    q = nc.gpsimd if h % 2 == 0 else nc.sync
                    q.dma_start(
                        out=XTA[t][32 * sl + 16 : 32 * sl + 17, cs:ce],
                        in_=xnab[h : h + 1, cs:ce],
                    )
            # -2 scale on the zy slots (in place, 32-aligned partition bases)
            for h in range(H):
                t, sl = HT[h], HS[h]
                nc.vector.tensor_scalar(
                    out=YTA[t][32 * sl : 32 * sl + 16, :],
                    in0=YTA[t][32 * sl : 32 * sl + 16, :],
                    scalar1=-2.0, scalar2=None, op0=ALU.mult,
                )
            # yn rows
            sqy = pp.tile([128, RPC], bf16, tag="sqy")
            nc.scalar.activation(out=sqy[:], in_=SYT[:], func=AF.Square)
            ynab = ppk.tile([8, RPC], bf16)
            for t in range(RPC // 512):
                ynp = ppp.tile([8, 512], f32, tag="xn")
                nc.tensor.matmul(
                    out=ynp[:], lhsT=Hmaskb[:, 0:8],
                    rhs=sqy[:, t * 512 : (t + 1) * 512],
                )
                nc.vector.tensor_scalar(
                    out=ynab[:, t * 512 : (t + 1) * 512], in0=ynp[:],
                    scalar1=half8[:], scalar2=None, op0=ALU.add,
                )
            for h in range(H):
                t, sl = HT[h], HS[h]
                q = nc.gpsimd if h % 2 == 0 else nc.sync
                q.dma_start(
                    out=YTA[t][32 * sl + 17 : 32 * sl + 18, :],
                    in_=ynab[h : h + 1, :],
                )


        def emit_diag():
            # diagonal path (f32): vd_h(i) = 1 + ||zy_i - zx_i||^2 per head;
            # emitted inside the loop so it fills DVE gaps instead of
            # blocking the first reciprocals at the queue head
            for t in range(NIB):
                dd = dgp.tile([128, 128], f32, tag="dd")
                nc.vector.tensor_sub(dd[:], SYD[:, t * 256 : t * 256 + 128],
                                     SYD[:, t * 256 + 128 : (t + 1) * 256])
                nc.vector.tensor_mul(dd[:], dd[:], dd[:])
                nc.vector.tensor_reduce(
                    out=vdall[:, t * H : (t + 1) * H],
                    in_=dd.rearrange("p (h k) -> p h k", k=DH),
                    axis=AX.X, op=ALU.add,
                )
            nc.vector.tensor_scalar(
                out=vdall[:], in0=vdall[:], scalar1=1.0, scalar2=None, op0=ALU.add
            )
            nc.vector.reciprocal_approx_fast(out=wdall[:], in_=vdall[:])
            wv = wdall.rearrange("p (t c) -> p t c", c=8)
            nc.vector.tensor_mul(
                pd1.rearrange("p (t c) -> p t c", c=4), wv[:, :, 0:4], wv[:, :, 4:8]
            )
            p1v = pd1.rearrange("p (t c) -> p t c", c=4)
            nc.vector.tensor_mul(
                pd2.rearrange("p (t c) -> p t c", c=2), p1v[:, :, 0:2], p1v[:, :, 2:4]
            )
            p2v = pd2.rearrange("p (t c) -> p t c", c=2)
            nc.vector.tensor_mul(
                pdw.rearrange("p (t c) -> p t c", c=1), p2v[:, :, 0:1], p2v[:, :, 1:2]
            )
            # Ld export (stats cols 0:4)
            nc.scalar.activation(out=stats[:, 0:4], in_=pdw[:], func=AF.Ln)
            # wdT: diag w transposed to [8, RPC] (for rowsum subtraction)
            for t in range(NIB):
                ptw = aux.tile([128, 128], f32, tag="aux")
                nc.tensor.transpose(
                    ptw[0:8, :], wdall[:, t * 8 : (t + 1) * 8], ident[:]
                )
                nc.scalar.activation(
                    out=wdT[:, t * 128 : (t + 1) * 128], in_=ptw[0:8, :],
                    func=AF.Copy,
                )

        # ---------- main loop (group-major: jb = 4k+g) ----------
        dgp = stk.enter_context(tc.tile_pool(name="diag_sbuf", bufs=2))
        rp = stk.enter_context(tc.tile_pool(name="rs_psum", bufs=1, space="PSUM"))
        aux = stk.enter_context(tc.tile_pool(name="aux_psum", bufs=1, space="PSUM"))
        rsAB = rp.tile([40, 512], f32)  # A at partitions 0:8, B at 32:40
        with (
            tc.tile_pool(name="mm_psum", bufs=2, space="PSUM") as mp,
            tc.tile_pool(name="wpool2", bufs=6) as wp2,
            tc.tile_pool(name="lnvp", bufs=5) as lvp,
            tc.tile_pool(name="upool", bufs=12) as up,
            tc.tile_pool(name="qpool", bufs=3) as qp,
            tc.tile_pool(name="lnqp", bufs=2) as lqp,
            tc.tile_pool(name="junkp", bufs=2) as jp,
        ):
            def sweeps(g, half, q0=0, q1=4):
                """Ln + sigmoid + count sweeps over chunks [q0,q1) of a half
                of Qst[g] (chunk = 512 cols)."""
                base = half * 4 * 512
                qs = Qst[g][:, base + q0 * 512 : base + q1 * 512]
                W = (q1 - q0) * 512
                lnq = lqp.tile([128, W], bf16, tag="lnq")
                if q0 == 0:
                    a_ln = (slq if half == 0 else slq2)[:, g : g + 1]
                    a_sg = (ssig if half == 0 else ssig2)[:, g : g + 1]
                    a_lo = (scnt if half == 0 else scnt2)[:, g : g + 1]
                    a_hi = (scnt if half == 0 else scnt2)[:, 4 + g : 5 + g]
                else:
                    aq = accq if q0 == 2 else accq2
                    a_ln = aq[:, g : g + 1]
                    a_sg = aq[:, 4 + g : 5 + g]
                    a_lo = aq[:, 8 + g : 9 + g]
                    a_hi = aq[:, 12 + g : 13 + g]
                nc.scalar.activation(
                    out=lnq[:], in_=qs, func=AF.Ln, accum_out=a_ln,
                )
                sj = jp.tile([128, W], bf16, tag="sj")
                nc.scalar.activation(
                    out=sj[:], in_=lnq[:], func=AF.Sigmoid, scale=1.0 / H,
                    bias=nbl[:], accum_out=a_sg,
                )
                cj = jp.tile([128, W], bf16, tag="sj")
                nc.scalar.activation(
                    out=cj[:], in_=qs, func=AF.Sign, bias=nthr_lo[:],
                    accum_out=a_lo,
                )
                ch = jp.tile([128, W], bf16, tag="sj")
                nc.scalar.activation(
                    out=ch[:], in_=qs, func=AF.Sign, bias=nthr_hi[:],
                    accum_out=a_hi,
                )

            for e in range(NJB):
                g, k = e // 8, e % 8
                jb = 4 * k + g
                acc = rsAB[0:8, :] if g == 0 else rsAB[32:40, :]
                w2t = []
                for r, heads in enumerate(RHEADS):
                    L = len(heads) * 512
                    w2 = wp2.tile([128, L], bf16, tag=f"w2{r}", name=f"w2_{r}")
                    ps = mp.tile([128, 1536], f32, tag="ps")
                    for si, h in enumerate(heads):
                        nc.tensor.matmul(
                            out=ps[:, si * 512 : (si + 1) * 512],
                            lhsT=XTA[r][32 * si : 32 * si + 18,
                                        jb * 128 : (jb + 1) * 128],
                            rhs=YTA[r][32 * si : 32 * si + 18, :],
                        )
                    nc.vector._custom_dve(
                        RECIPROCAL_APPROX_FAST,
                        out=w2[:], in0=ps[:, 0:L],
                        s0=RC["s0"], s1=RC["s1"], imm2=RC["imm2"],
                    )
                    # per-(i,h) row sums accumulate on the PE
                    for si, h in enumerate(heads):
                        nc.tensor.matmul(
                            out=acc[:],
                            lhsT=Eall[:, h * 8 : (h + 1) * 8],
                            rhs=w2[:, si * 512 : (si + 1) * 512],
                            start=(e in (0, 8) and h == 0),
                            stop=(e in (7, 31) and h == H - 1),
                            skip_group_check=True,
                        )
                    w2t.append(w2)
                # product tree over the 8 heads in bf16
                pairs = [
                    (w2t[0][:, 0:512], w2t[0][:, 512:1024]),      # h0*h1
                    (w2t[0][:, 1024:1536], w2t[1][:, 0:512]),     # h2*h3
                    (w2t[1][:, 512:1024], w2t[1][:, 1024:1536]),  # h4*h5
                    (w2t[2][:, 0:512], w2t[2][:, 512:1024]),      # h6*h7
                ]
                us = []
                for pi, (a, b) in enumerate(pairs):
                    u = up.tile([128, 512], bf16, tag="u", name=f"u{pi}")
                    if pi < 2:
                        nc.vector.tensor_mul(u[:], a, b)
                    else:
                        nc.gpsimd.tensor_mul(u[:], a, b)
                    us.append(u)
                qa = qp.tile([128, 512], bf16, tag="q")
                qb = qp.tile([128, 512], bf16, tag="q")
                nc.gpsimd.tensor_mul(qa[:], us[0][:], us[1][:])
                nc.gpsimd.tensor_mul(qb[:], us[2][:], us[3][:])
                nc.gpsimd.tensor_mul(
                    Qst[g][:, k * 512 : (k + 1) * 512], qa[:], qb[:]
                )
                if e == 2:
                    emit_diag()
                if e == 7:
                    # bl0 estimate from accumulator A (j-blocks {0,4,..,28};
                    # diag elements only for local rows 0:127)
                    nc.vector.tensor_reduce(
                        out=SA[:], in_=rsAB[0:8, :], axis=AX.X, op=ALU.add
                    )
                    nc.vector.tensor_reduce(
                        out=wdpart[:], in_=wdT[:, 0:128], axis=AX.X, op=ALU.add
                    )
                    nc.vector.tensor_sub(SA[:], SA[:], wdpart[:])
                    nc.scalar.activation(out=lnSA[:], in_=SA[:], func=AF.Ln)
                    ps1 = aux.tile([1, 1], f32, tag="aux")
                    nc.tensor.matmul(out=ps1[:], lhsT=ones8[:, 0:1], rhs=lnSA[:])
                    nc.scalar.activation(
                        out=bl0[:], in_=ps1[:], func=AF.Copy, scale=1.0 / H,
                        bias=-LOG_A_PAIRS,
                    )
                    # bracket thresholds exp(H*bl0 -/+ eps); host interpolates
                    # the count at the exact baseline between them
                    nc.scalar.activation(
                        out=thr0[:, 0:1], in_=bl0[:], func=AF.Exp, scale=float(H),
                        bias=epslo[:],
                    )
                    nc.scalar.activation(
                        out=thr0[:, 1:2], in_=bl0[:], func=AF.Exp, scale=float(H),
                        bias=epshi[:],
                    )
                    psB_ = aux.tile([128, 1], f32, tag="aux")
                    nc.tensor.matmul(out=psB_[:], lhsT=ones1[0:1, :], rhs=bl0[0:1, :])
                    nc.scalar.activation(
                        out=nbl[:], in_=psB_[:], func=AF.Copy, scale=-1.0
                    )
                    psT_ = aux.tile([128, 2], f32, tag="aux")
                    nc.tensor.matmul(out=psT_[:], lhsT=ones1[0:1, :], rhs=thr0[0:1, :])
                    nc.scalar.activation(
                        out=nthr_lo[:], in_=psT_[:, 0:1], func=AF.Copy, scale=-1.0
                    )
                    nc.scalar.activation(
                        out=nthr_hi[:], in_=psT_[:, 1:2], func=AF.Copy, scale=-1.0
                    )
                    # SA export + bl0 export (host reproduces the thresholds)
                    nc.vector.tensor_copy(stats[0:8, 20:21], SA[:])
                    nc.vector.tensor_copy(stats[0:1, 21:22], bl0[:])
                    # pre-subtract: rsS = A - wdT (tail just adds B)
                    nc.scalar.activation(out=rsS[:], in_=rsAB[0:8, :], func=AF.Copy)
                    nc.vector.tensor_sub(rsS[:], rsS[:], wdT[:])
                if k == 3 and e >= 11:
                    sweeps(g, 0)
                if e == 29:
                    sweeps(3, 1, 0, 2)   # quarter/eighth splits: last group
                if e == 30:
                    sweeps(3, 1, 2, 3)
                if k == 7:
                    if e == 7:
                        sweeps(0, 0)
                    if e == 31:
                        sweeps(3, 1, 3, 4)
                    else:
                        sweeps(g, 1)

        # ---------- finish ----------
        with (
            tc.tile_pool(name="fin_sbuf", bufs=2) as fs,
        ):
            # rsS already holds A - wdT; add B
            nc.vector.tensor_add(rsS[:], rsS[:], rsAB[32:40, :])
            # exports
            nc.sync.dma_start(out=out_rs[:], in_=rsS[:])
            nc.vector.tensor_add(stats[:, 4:8], slq[:], slq2[:])
            nc.vector.tensor_add(stats[:, 4:8], stats[:, 4:8], accq[:, 0:4])
            nc.vector.tensor_add(stats[:, 4:8], stats[:, 4:8], accq2[:, 0:4])
            nc.vector.tensor_add(stats[:, 8:12], ssig[:], ssig2[:])
            nc.vector.tensor_add(stats[:, 8:12], stats[:, 8:12], accq[:, 4:8])
            nc.vector.tensor_add(stats[:, 8:12], stats[:, 8:12], accq2[:, 4:8])
            nc.vector.tensor_add(stats[:, 12:16], scnt[:, 0:4], scnt2[:, 0:4])
            nc.vector.tensor_add(stats[:, 12:16], stats[:, 12:16], accq[:, 8:12])
            nc.vector.tensor_add(stats[:, 12:16], stats[:, 12:16], accq2[:, 8:12])
            nc.vector.tensor_add(stats[:, 16:20], scnt[:, 4:8], scnt2[:, 4:8])
            nc.vector.tensor_add(stats[:, 16:20], stats[:, 16:20], accq[:, 12:16])
            nc.vector.tensor_add(stats[:, 16:20], stats[:, 16:20], accq2[:, 12:16])
            nc.sync.dma_start(out=out[:], in_=stats[:])

    nc.compile()
    return nc


_CACHED_NC = None


def _get_nc():
    global _CACHED_NC
    if _CACHED_NC is None:
        _CACHED_NC = build_bass()
    return _CACHED_NC


_HT = [0, 0, 0, 1, 1, 1, 2, 2]
_HS = [0, 1, 2, 0, 1, 2, 0, 1]


def make_in_maps(z_x, z_y):
    """Host-side prep is layout + dtype only: shard, transpose, and place the
    z rows into the 32-row matmul slots (constant one-rows prefilled, norm
    rows zeroed -- the device computes all arithmetic: -2 scale, norms)."""
    import ml_dtypes

    bf = ml_dtypes.bfloat16
    z_x32 = np.ascontiguousarray(z_x, dtype=np.float32)
    z_y32 = np.ascontiguousarray(z_y, dtype=np.float32)
    z_xt = np.ascontiguousarray(z_x32.astype(bf).T)
    z_yt_full = z_y32.astype(bf).T
    xta = [np.zeros((96, N), bf) for t in range(3)]
    for h in range(H):
        t, s = _HT[h], _HS[h]
        xta[t][32 * s : 32 * s + 16] = z_xt[DH * h : DH * (h + 1)]
        xta[t][32 * s + 17] = np.ones((N,), bf)
    maps = []
    for c in range(NCORES):
        z_yt = np.ascontiguousarray(z_yt_full[:, c * RPC : (c + 1) * RPC])
        yta = [np.zeros((96, RPC), bf) for t in range(3)]
        for h in range(H):
            t, s = _HT[h], _HS[h]
            yta[t][32 * s : 32 * s + 16] = z_yt[DH * h : DH * (h + 1)]
            yta[t][32 * s + 16] = np.ones((RPC,), bf)
        maps.append(
            {
                "z_xt": z_xt,
                "z_yt": z_yt,
                "xta0": xta[0], "xta1": xta[1], "xta2": xta[2],
                "yta0": yta[0], "yta1": yta[1], "yta2": yta[2],
                "z_yd": np.ascontiguousarray(
                    np.concatenate(
                        [
                            z_y32[c * RPC : (c + 1) * RPC],
                            z_x32[c * RPC : (c + 1) * RPC],
                        ],
                        axis=1,
                    )
                ),
            }
        )
    return maps


def combine(stats, z_x, z_y):
    """stats: [NCORES, 128, NSTAT] float; returns the 9 reference outputs.

    stats = (st [NCORES,128,17], rs_all [NCORES,8,RPC]).
    st cols 0:4 Ld[p,t]; 4:8 slq; 8:12 ssig; 12:16 scnt;
    col 16 parts 0:8 = SA_h (partial sums the device used for bl0/thr0).
    """
    st, rs_all = stats
    st = st.astype(np.float64)
    Ld = st[:, :, 0:4]                       # [c, p, t]
    slq = st[:, :, 4:8].sum()
    ssig = st[:, :, 8:12].sum()
    swept = 128.0 * 4096.0 * 4                  # values per core per bracket
    cnt_lo = (st[:, :, 12:16].sum(axis=(1, 2)) + swept) / 2.0  # [c]
    cnt_hi = (st[:, :, 16:20].sum(axis=(1, 2)) + swept) / 2.0  # [c]
    rs = rs_all.astype(np.float64)           # [c, h, i]
    SA = st[:, 0:8, 20]                      # [c, h]
    bl0_dev = st[:, 0, 21]                   # [c] the bl0 each device used

    S_h = rs.sum(axis=(0, 2))                # [h]
    blavg = np.log(S_h).mean() - math.log(float(N) * (N - 1))
    rep_sum = np.log(rs).sum()

    sum_Ld = Ld.sum()
    Ld_flat = Ld.reshape(NCORES, -1)
    # exact diag stats with exact blavg
    cp = float((Ld_flat / H - blavg > 0).sum())
    sig_diag = (1.0 / (1.0 + np.exp(-(Ld_flat / H - blavg)))).sum()
    # device-side sigmoid model (bl0 per core, exported)
    sig_diag_dev = 0.0
    for c in range(NCORES):
        sig_diag_dev += (1.0 / (1.0 + np.exp(-(Ld_flat[c] / H - bl0_dev[c])))).sum()
    # count at the EXACT threshold t=8*blavg by linear interpolation between
    # the two bracketing device sweeps at 8*bl0_dev -/+ CNT_EPS
    t_true = H * blavg
    t_lo = H * bl0_dev - CNT_EPS             # [c]
    frac = (t_true - t_lo) / (2.0 * CNT_EPS)
    cnt_at_true = cnt_lo + (cnt_hi - cnt_lo) * frac  # counts decrease in t
    cnt_full = cnt_at_true.sum()

    mean_pos = sum_Ld / (H * N) - blavg
    mean_neg = (slq - sum_Ld) / (H * N * (N - 1)) - blavg
    mean_sig_pos = sig_diag / N
    mean_sig_neg = (ssig - sig_diag_dev) / (N * (N - 1))
    cn = cnt_full - cp
    acc = (cp + (N * (N - 1) - cn)) / (N * N)
    recall = cp / N
    tpfp = cp + cn
    precision = (cp / max(tpfp, 1.0)) if tpfp > 0 else 0.0
    rep_mean = rep_sum / (H * N) - math.log(N - 1) - blavg
    zx64 = np.asarray(z_x, np.float64)
    zy64 = np.asarray(z_y, np.float64)
    decay = 0.01 * (np.mean(zx64 * zx64) + np.mean(zy64 * zy64))
    loss = -mean_pos + rep_mean + decay
    return np.array(
        [
            mean_pos, mean_neg, mean_sig_pos, mean_sig_neg, acc, recall,
            precision, blavg, loss,
        ],
        dtype=np.float32,
    )


def run_on_hw(z_x, z_y, trace=False):
    from concourse.bass_utils import run_bass_kernel_spmd

    nc = _get_nc()
    res = run_bass_kernel_spmd(
        nc, make_in_maps(z_x, z_y), core_ids=list(range(NCORES)), trace=trace
    )
    st = np.stack([np.asarray(r["out"]) for r in res.results])
    rs_all = np.stack([np.asarray(r["out_rs"]) for r in res.results])
    return combine((st, rs_all), z_x, z_y), res


def kernel(z_x, z_y):
    out, _ = run_on_hw(z_x, z_y, trace=False)
    return out
